# revision 1
# baseline (speedup 1.0000x reference)
"""Trainium2 Bass kernel for nn_CombinedLoss (regression MSE + masked binary focal loss).

Data-parallel over 8 NeuronCores: each core reduces its batch shard to
per-class partial sums; the final (tiny) weighted combination happens on host
in float64.

Math (per element of the 13 presence classes, t in {-1, 0, 1}):
    z  = (1 - 2t) * x          (so z = x for t=0, -x for t=1)
    focal(x, t) = softplus(z) * sigmoid(z)^2      for valid t (t != -1)
    weighted by w_c if t==1 else (1-w_c), masked out for t==-1.

On-device we avoid masking entirely by reducing three per-class sums over ALL
elements (including t==-1 garbage, which is finite):
    S0_c  = sum f          Sh_c  = sum f*t        Sh2_c = sum f*t^2
Host side:   F1 = (Sh+Sh2)/2  (t==1 sum),  F0 = S0-Sh2  (t==0 sum)
    focal_total = sum_c (1-w_c)*F0_c + w_c*F1_c

softplus/sigmoid use only the `natural_log_exp_and_others` ACT table set
(softplus has no HW table on this toolchain):
    e  = exp(z)            [ACT Exp]
    sp = ln(e + 1)         [ACT Ln,  free bias=+1]  == softplus(z), bf16
    s2 = exp(2*(z - sp))   [ACT Exp, scale=+2]      == sigmoid(z)^2, bf16
so the ACT engine never switches table sets (one ACT_TABLE_LOAD total).

Steady state is DMA-bound (~101us/core roofline); measured ~122us/iter via
the in-NEFF repeat-loop slope. Engine balance per tile: DMA 3.2us,
DVE ~2.9us (a', v, f, h, h2, d, q), ACT ~3.1us (e, sp, s2, 2/3 of tp),
GPSIMD ~2.5us (z, 1/3 of tp), PE ~1.6us (7 accumulating matmuls).

The per-class reductions run on the (otherwise idle) TensorEngine as
ones-vector matmuls accumulating into PSUM in fp32.
"""

import sys

if "/opt/trn_rl_repo" not in sys.path:
    sys.path.insert(0, "/opt/trn_rl_repo")

import numpy as np

NCORES = 8
B = 2_097_152
BS = B // NCORES          # 262144 rows per core
P = 128                   # SBUF partitions
RPP = BS // P             # 2048 rows per partition
T = 64                    # rows per tile
NT = RPP // T             # 32 tiles
G = 2                     # tiles per group (one elementwise op instr per group)
NGROUP = NT // G          # 16 groups
FD_FULL = T * 16          # 1024 fp32 per partition per tile (4KB DMA run)
FD_C = T * 13             # 832 class elements per partition per tile
FD_R = T * 3              # 192 regression elements per partition per tile
NPART = 3 * FD_C + FD_R   # 2688 partial-sum cells per core


def build(reps: int = 1, g: int = 1, bufs_io: int = 8, bufs32: int = 8,
          bufs16: int = 8, dma_split: int = 1, plan: str = "az",
          tp_alt: int = 3, bufs_tx: int = 3):
    import concourse.bacc as bacc
    import concourse.mybir as mybir
    import concourse.tile as tile
    import bass_rust as _bass_rust
    from concourse.hw_specs import get_activation_tables

    dt = mybir.dt
    AF = mybir.ActivationFunctionType
    OP = mybir.AluOpType

    class _Bacc(bacc.Bacc):
        """Pin every activation to the natural_log_exp_and_others table set.

        The default chooser scans act_func_sets in order and picks the first
        set containing each function, so Exp lands in exp_and_others and Ln
        in natural_log — alternating ACT_TABLE_LOADs (~1.3us each) every
        group. All functions this kernel uses (Exp, Ln, Copy) live together
        in natural_log_exp_and_others; blanking the other sets (positions
        preserved, since act_func_set_id is the list index) yields exactly
        one table load for the whole kernel.
        """

        def insert_act_table_loads(self):
            has_activation = any(
                isinstance(i, mybir.InstActivation)
                for b in self.main_func.blocks
                for i in b.instructions
            )
            if not has_activation:
                return
            keep = "natural_log_exp_and_others"
            tables = [
                (name, funcs if name == keep else set())
                for name, funcs in get_activation_tables(self.m.arch).items()
            ]
            _bass_rust.insert_act_table_loads(self, tables)

    G_ = g
    NGROUP_ = NT // G_
    FDGF = G_ * FD_FULL
    FDGC = G_ * FD_C
    FDGR = G_ * FD_R

    nc = _Bacc("TRN2", target_bir_lowering=False, debug=False,
               num_devices=NCORES)
    x_d = nc.dram_tensor("output", [BS, 16], dt.float32, kind="ExternalInput")
    t_d = nc.dram_tensor("target", [BS, 16], dt.float32, kind="ExternalInput")
    po_d = nc.dram_tensor("partials", [1, NPART], dt.float32,
                          kind="ExternalOutput")

    # [128, 32768] per-partition contiguous row blocks
    xv = x_d.ap().rearrange("(p r) c -> p (r c)", p=P)
    tv = t_d.ap().rearrange("(p r) c -> p (r c)", p=P)

    with tile.TileContext(nc) as tc:
        with (
            tc.tile_pool(name="io", bufs=bufs_io) as io_pool,
            tc.tile_pool(name="f32", bufs=bufs32) as f32_pool,
            tc.tile_pool(name="b16", bufs=bufs16) as b16_pool,
            tc.tile_pool(name="tx", bufs=bufs_tx) as tx_pool,
            tc.tile_pool(name="cst", bufs=1) as cst_pool,
            tc.tile_pool(name="acc", bufs=1, space="PSUM") as psum_pool,
        ):
            ones = cst_pool.tile([P, 1], dt.bfloat16, tag="ones")
            nc.vector.memset(ones[:], 1.0)

            p0 = psum_pool.tile([1, FD_C], dt.float32, tag="p0")
            p1 = psum_pool.tile([1, FD_C], dt.float32, tag="p1")
            p2 = psum_pool.tile([1, FD_C], dt.float32, tag="p2")
            pq = psum_pool.tile([1, FD_R], dt.float32, tag="pq")

            for rep in range(reps):
                for gi in range(NGROUP_):
                    xg = io_pool.tile([P, FDGF], dt.float32, tag="xg")
                    tg = io_pool.tile([P, FDGF], dt.float32, tag="tg")
                    # dma_split = number of DMAs per tensor per group
                    step = FDGF // dma_split
                    for i in range(dma_split):
                        sl_s = slice(i * step, (i + 1) * step)
                        sl_d = slice(gi * FDGF + i * step,
                                     gi * FDGF + (i + 1) * step)
                        nc.sync.dma_start(xg[:, sl_s], xv[:, sl_d])
                        nc.sync.dma_start(tg[:, sl_s], tv[:, sl_d])

                    x3 = xg[:].rearrange("p (r c) -> p r c", c=16)
                    t3 = tg[:].rearrange("p (r c) -> p r c", c=16)
                    xc, tc_v = x3[:, :, 3:16], t3[:, :, 3:16]
                    xr, tr_v = x3[:, :, 0:3], t3[:, :, 0:3]

                    zg = f32_pool.tile([P, FDGC], dt.float32, tag="z")
                    if plan == "az":
                        # a = 1 - 2t; z = x * a  (z on GPSIMD)
                        ag = tx_pool.tile([P, FDGC], dt.float32, tag="a")
                        nc.vector.tensor_scalar(
                            ag[:], tc_v, -2.0, 1.0, OP.mult, OP.add)
                        nc.gpsimd.tensor_tensor(zg[:], xc, ag[:], OP.mult)
                    else:
                        # z' = (t - 0.5) x; ACT applies z = -2 z'
                        nc.vector.scalar_tensor_tensor(
                            zg[:], tc_v, 0.5, xc, OP.subtract, OP.mult)

                    # packed bf16 copy of class targets (exact for -1/0/1);
                    # alternate Pool/ACT per group to balance engine load
                    tpg = b16_pool.tile([P, FDGC], dt.bfloat16, tag="tp")
                    if tp_alt and gi % tp_alt != 0:
                        nc.scalar.activation(tpg[:], tc_v, AF.Copy)
                    else:
                        nc.gpsimd.tensor_copy(tpg[:], tc_v)

                    # regression: q = (x - t)^2 in bf16 (square on ACT)
                    dg = tx_pool.tile([P, FDGR], dt.bfloat16, tag="d")
                    nc.vector.tensor_tensor(dg[:], xr, tr_v, OP.subtract)
                    qg = tx_pool.tile([P, FDGR], dt.bfloat16, tag="q")
                    nc.vector.tensor_tensor(qg[:], dg[:], dg[:], OP.mult)

                    # e = exp(z)
                    zscale = 1.0 if plan == "az" else -2.0
                    eg = f32_pool.tile([P, FDGC], dt.float32, tag="scr")
                    nc.scalar.activation(eg[:], zg[:], AF.Exp, scale=zscale)
                    # sp = ln(e + 1) = softplus(z), rounded to bf16
                    spg = b16_pool.tile([P, FDGC], dt.bfloat16, tag="sp")
                    nc.scalar.activation(spg[:], eg[:], AF.Ln, bias=1.0)
                    # v = z - sp  (mixed fp32/bf16)
                    vg = f32_pool.tile([P, FDGC], dt.float32, tag="scr")
                    if plan == "az":
                        nc.vector.tensor_tensor(vg[:], zg[:], spg[:],
                                                OP.subtract)
                    else:
                        nc.vector.scalar_tensor_tensor(
                            vg[:], zg[:], -2.0, spg[:], OP.mult, OP.subtract)
                    # s2 = exp(2 v) = sigmoid(z)^2
                    s2g = b16_pool.tile([P, FDGC], dt.bfloat16, tag="s2")
                    nc.scalar.activation(s2g[:], vg[:], AF.Exp, scale=2.0)

                    fg = tx_pool.tile([P, FDGC], dt.bfloat16, tag="f")
                    nc.vector.tensor_tensor(fg[:], spg[:], s2g[:], OP.mult)
                    hg = tx_pool.tile([P, FDGC], dt.bfloat16, tag="h")
                    nc.vector.tensor_tensor(hg[:], fg[:], tpg[:], OP.mult)
                    h2g = tx_pool.tile([P, FDGC], dt.bfloat16, tag="h2")
                    nc.vector.tensor_tensor(h2g[:], hg[:], tpg[:], OP.mult)

                    for i in range(G_):
                        j = gi * G_ + i
                        st = j == 0
                        fin = j == NT - 1
                        off = i * FD_C
                        for (acc, src) in ((p0, fg), (p1, hg), (p2, h2g)):
                            nc.tensor.matmul(acc[:, 0:512], ones[:],
                                             src[:, off:off + 512],
                                             start=st, stop=fin)
                            nc.tensor.matmul(acc[:, 512:FD_C], ones[:],
                                             src[:, off + 512:off + FD_C],
                                             start=st, stop=fin)
                        nc.tensor.matmul(pq[:], ones[:],
                                         qg[:, i * FD_R:(i + 1) * FD_R],
                                         start=st, stop=fin)

            outt = cst_pool.tile([1, NPART], dt.float32, tag="out")
            nc.scalar.copy(outt[:, 0:FD_C], p0[:])
            nc.scalar.copy(outt[:, FD_C:2 * FD_C], p1[:])
            nc.scalar.copy(outt[:, 2 * FD_C:3 * FD_C], p2[:])
            nc.scalar.copy(outt[:, 3 * FD_C:NPART], pq[:])
            nc.sync.dma_start(po_d.ap(), outt[:])

    nc.compile()
    return nc


# ---------------------------------------------------------------------------
# Cached PJRT executor (jit once per process; later calls are cheap).
# Mirrors concourse.bass2jax.run_bass_via_pjrt for the 8-core SPMD case.
# ---------------------------------------------------------------------------

_EXEC = None


def _get_executor():
    global _EXEC
    if _EXEC is not None:
        return _EXEC

    import jax
    import concourse.mybir as mybir
    from concourse import bass2jax
    from jax.sharding import Mesh, PartitionSpec
    from jax.experimental.shard_map import shard_map

    nc = build(1)
    bass2jax.install_neuronx_cc_hook()

    partition_name = (nc.partition_id_tensor.name
                      if nc.partition_id_tensor else None)
    in_names, out_names, out_avals = [], [], []
    for alloc in nc.m.functions[0].allocations:
        if not isinstance(alloc, mybir.MemoryLocationSet):
            continue
        name = alloc.memorylocations[0].name
        if alloc.kind == "ExternalInput":
            if name != partition_name:
                in_names.append(name)
        elif alloc.kind == "ExternalOutput":
            out_names.append(name)
            out_avals.append(jax.core.ShapedArray(
                tuple(alloc.tensor_shape), mybir.dt.np(alloc.dtype)))

    n_params = len(in_names)
    n_outs = len(out_avals)
    all_in_names = list(in_names) + list(out_names)
    if partition_name is not None:
        all_in_names.append(partition_name)

    def _body(*args):
        operands = list(args)
        if partition_name is not None:
            operands.append(bass2jax.partition_id_tensor())
        return tuple(bass2jax._bass_exec_p.bind(
            *operands,
            out_avals=tuple(out_avals),
            in_names=tuple(all_in_names),
            out_names=tuple(out_names),
            lowering_input_output_aliases=(),
            sim_require_finite=True,
            sim_require_nnan=True,
            nc=nc,
        ))

    devices = jax.devices()[:NCORES]
    mesh = Mesh(np.asarray(devices), ("core",))
    in_specs = (PartitionSpec("core"),) * (n_params + n_outs)
    out_specs = (PartitionSpec("core"),) * n_outs
    donate = tuple(range(n_params, n_params + n_outs))
    sharded = jax.jit(
        shard_map(_body, mesh=mesh, in_specs=in_specs, out_specs=out_specs,
                  check_rep=False),
        donate_argnums=donate, keep_unused=True)

    _EXEC = (sharded, in_names, out_names, out_avals)
    return _EXEC


def run_device_partials(output: np.ndarray, target: np.ndarray) -> np.ndarray:
    """Run the SPMD kernel; returns per-core partials [NCORES, NPART] fp32."""
    sharded, in_names, out_names, out_avals = _get_executor()
    feeds = {"output": np.ascontiguousarray(output, dtype=np.float32),
             "target": np.ascontiguousarray(target, dtype=np.float32)}
    ins = [feeds[n] for n in in_names]
    zeros = [np.zeros((NCORES * a.shape[0],) + a.shape[1:], a.dtype)
             for a in out_avals]
    outs = sharded(*ins, *zeros)
    idx = out_names.index("partials")
    return np.asarray(outs[idx]).reshape(NCORES, NPART)


def combine_partials(partials: np.ndarray,
                     binary_class_weights: np.ndarray) -> np.float32:
    """Host-side fp64 combination of per-core partial sums into the loss."""
    p = partials.astype(np.float64).sum(axis=0)
    S0 = p[0:FD_C].reshape(T, 13).sum(axis=0)
    Sh = p[FD_C:2 * FD_C].reshape(T, 13).sum(axis=0)
    Sh2 = p[2 * FD_C:3 * FD_C].reshape(T, 13).sum(axis=0)
    Q = p[3 * FD_C:NPART].reshape(T, 3).sum(axis=0)
    w = np.asarray(binary_class_weights, dtype=np.float64)
    F1 = (Sh + Sh2) / 2.0
    F0 = S0 - Sh2
    focal = np.sum((1.0 - w) * F0 + w * F1)
    mse = Q / float(B)
    loss = 10.0 * mse[0] + mse[1] + mse[2] + focal
    return np.float32(loss)


def kernel(output: np.ndarray, target: np.ndarray,
           binary_class_weights: np.ndarray) -> np.ndarray:
    partials = run_device_partials(output, target)
    return np.asarray(combine_partials(partials, binary_class_weights))



# revision 2
# speedup vs baseline: 1.4019x; 1.4019x over previous
"""Trainium2 Bass kernel for nn_CombinedLoss (regression MSE + masked binary
focal loss), data-parallel over 8 NeuronCores.

v4: all-bf16 compute fed by SWDGE cast-DMA.
 - nc.gpsimd.dma_start casts f32->bf16 in the DMA datapath: HBM read bytes
   unchanged (32 MiB/core, the roofline), SBUF writes halved. GPSIMD does
   NOTHING else: its tensor ops lock the shared DVE<->GpSimd SBUF port and
   double DVE op cost (measured).
 - All elementwise ops on DVE at 2x bf16 rate, plain tensor_tensor /
   tensor_scalar only (scalar_tensor_tensor measured 2.4x slower than TT).
 - ACT chain uses the negated form so every scalar coefficient folds into
   activation scale/bias: em=exp(-z), spm=ln(em+1)=softplus(-z),
   s2=exp(-2 spm)=sigmoid(z)^2, then sp=z+spm, f=sp*s2.
 - DVE pre-reduces the G=2 subtiles (fR/hR/h2R pair-adds) before the PE
   ones-matmul reduction: PE streams 1 column/cycle, DVE is ~2.7x faster
   per element, so this halves PE time.
 - Per-class sums S0=sum f, Sh=sum f*t, Sh2=sum f*t^2 accumulate in PSUM
   f32; host combines in f64: F1=(Sh+Sh2)/2, F0=S0-Sh2 (t=-1 garbage
   cancels exactly; *(+-1) multiplies are exact in bf16).

Measured (interleaved A/B, reps=257 in-NEFF loop slope, median): ~88-100us
vs 187us for the f32 v1 baseline under the same method. DMA floor ~86-94us
(32 MiB/core at ~358-390 GB/s effective).
"""

import sys

if "/opt/trn_rl_repo" not in sys.path:
    sys.path.insert(0, "/opt/trn_rl_repo")

import numpy as np

NCORES = 8
B = 2_097_152
BS = B // NCORES          # 262144 rows per core
P = 128                   # SBUF partitions
RPP = BS // P             # 2048 rows per partition
T = 64                    # rows per subtile (psum accumulation granularity)
NT = RPP // T             # 32 subtiles
FD_FULL = T * 16          # 1024 elems per partition per subtile
FD_C = T * 13             # 832 class elements per partition per subtile
FD_R = T * 3              # 192 regression elements
G_FIX = 2                 # group size (pq layout depends on it)
NPART = 3 * FD_C + G_FIX * FD_R   # 2880 partial-sum cells per core


def build(reps: int = 1, g: int = 2, bufs_io: int = 6, bufs_w: int = 3,
          bufs_s: int = 3, cast: int = 1, contig: int = 0):
    import concourse.bacc as bacc
    import concourse.mybir as mybir
    import concourse.tile as tile
    import bass_rust as _bass_rust
    from concourse.hw_specs import get_activation_tables

    dt = mybir.dt
    AF = mybir.ActivationFunctionType
    OP = mybir.AluOpType

    class _Bacc(bacc.Bacc):
        """Pin every activation to natural_log_exp_and_others (one table
        load total; Exp/Ln/Copy all live there)."""

        def insert_act_table_loads(self):
            has_activation = any(
                isinstance(i, mybir.InstActivation)
                for b in self.main_func.blocks
                for i in b.instructions
            )
            if not has_activation:
                return
            keep = "natural_log_exp_and_others"
            tables = [
                (name, funcs if name == keep else set())
                for name, funcs in get_activation_tables(self.m.arch).items()
            ]
            _bass_rust.insert_act_table_loads(self, tables)

    G_ = g
    NGROUP_ = NT // G_
    FDGF = G_ * FD_FULL
    FDGC = G_ * FD_C
    FDGR = G_ * FD_R

    nc = _Bacc("TRN2", target_bir_lowering=False, debug=False,
               num_devices=NCORES)
    x_d = nc.dram_tensor("output", [BS, 16], dt.float32, kind="ExternalInput")
    t_d = nc.dram_tensor("target", [BS, 16], dt.float32, kind="ExternalInput")
    po_d = nc.dram_tensor("partials", [1, NPART], dt.float32,
                          kind="ExternalOutput")

    # [128, 32768]: partition p owns rows [p*RPP, (p+1)*RPP), contiguous
    xv = x_d.ap().rearrange("(p r) c -> p (r c)", p=P)
    tv = t_d.ap().rearrange("(p r) c -> p (r c)", p=P)

    with tile.TileContext(nc) as tc:
        with (
            tc.tile_pool(name="io", bufs=bufs_io) as io_pool,
            tc.tile_pool(name="wk", bufs=bufs_w) as wk_pool,
            tc.tile_pool(name="sm", bufs=bufs_s) as sm_pool,
            tc.tile_pool(name="cst", bufs=1) as cst_pool,
            tc.tile_pool(name="acc", bufs=1, space="PSUM") as psum_pool,
        ):
            ones = cst_pool.tile([P, 1], dt.bfloat16, tag="ones")
            nc.vector.memset(ones[:], 1.0)

            assert G_ == G_FIX, "pq layout hardcoded for g=2"
            p0 = psum_pool.tile([1, FD_C], dt.float32, tag="p0")
            p1 = psum_pool.tile([1, FD_C], dt.float32, tag="p1")
            p2 = psum_pool.tile([1, FD_C], dt.float32, tag="p2")
            pq = psum_pool.tile([1, G_ * FD_R], dt.float32, tag="pq")

            for rep in range(reps):
                for gi in range(NGROUP_):
                    sl = slice(gi * FDGF, (gi + 1) * FDGF)
                    xb = io_pool.tile([P, FDGF], dt.bfloat16, tag="xb")
                    tb = io_pool.tile([P, FDGF], dt.bfloat16, tag="tb")
                    # SWDGE cast-DMA: f32 HBM read, bf16 SBUF write
                    nc.gpsimd.dma_start(xb[:], xv[:, sl])
                    nc.gpsimd.dma_start(tb[:], tv[:, sl])

                    x3 = xb[:].rearrange("p (r c) -> p r c", c=16)
                    t3 = tb[:].rearrange("p (r c) -> p r c", c=16)
                    xc, tcv = x3[:, :, 3:16], t3[:, :, 3:16]
                    xr, trv = x3[:, :, 0:3], t3[:, :, 0:3]

                    # a = 1 - 2t (exact in bf16); z = x * a  -- no STT ops,
                    # all scalar coefficients fold into ACT scale/bias.
                    # Everything stays on DVE: gpsimd tensor ops lock the
                    # shared DVE<->GpSimd SBUF port and double DVE op cost.
                    ag = wk_pool.tile([P, FDGC], dt.bfloat16, tag="a")
                    nc.vector.tensor_scalar(ag[:], tcv, -2.0, 1.0,
                                            OP.mult, OP.add)
                    zg = wk_pool.tile([P, FDGC], dt.bfloat16, tag="z")
                    nc.vector.tensor_tensor(zg[:], xc, ag[:], OP.mult)
                    # em = exp(-z)
                    eg = wk_pool.tile([P, FDGC], dt.bfloat16, tag="e")
                    nc.scalar.activation(eg[:], zg[:], AF.Exp, scale=-1.0)
                    # spm = ln(em + 1) = softplus(-z) = sp - z
                    spm = wk_pool.tile([P, FDGC], dt.bfloat16, tag="spm")
                    nc.scalar.activation(spm[:], eg[:], AF.Ln, bias=1.0)
                    # s2 = exp(-2*spm) = sigmoid(z)^2
                    s2g = wk_pool.tile([P, FDGC], dt.bfloat16, tag="s2")
                    nc.scalar.activation(s2g[:], spm[:], AF.Exp, scale=-2.0)
                    # sp = z + spm = softplus(z)
                    spg = wk_pool.tile([P, FDGC], dt.bfloat16, tag="sp")
                    nc.vector.tensor_tensor(spg[:], zg[:], spm[:], OP.add)

                    fg = wk_pool.tile([P, FDGC], dt.bfloat16, tag="f")
                    nc.vector.tensor_tensor(fg[:], spg[:], s2g[:], OP.mult)
                    hg = wk_pool.tile([P, FDGC], dt.bfloat16, tag="h")
                    nc.vector.tensor_tensor(hg[:], fg[:], tcv, OP.mult)
                    h2g = wk_pool.tile([P, FDGC], dt.bfloat16, tag="h2")
                    nc.vector.tensor_tensor(h2g[:], hg[:], tcv, OP.mult)

                    # regression: q = (x - t)^2
                    dg = sm_pool.tile([P, FDGR], dt.bfloat16, tag="d")
                    nc.vector.tensor_tensor(dg[:], xr, trv, OP.subtract)
                    qg = sm_pool.tile([P, FDGR], dt.bfloat16, tag="q")
                    nc.vector.tensor_tensor(qg[:], dg[:], dg[:], OP.mult)

                    # DVE pre-reduction: fold the G=2 subtiles into one
                    # [P, FD_C] tensor each before PE. DVE is ~2.7x faster
                    # per element than PE's 1 column/cycle, so this halves
                    # the matmul column count.
                    fR = wk_pool.tile([P, FD_C], dt.bfloat16, tag="fR")
                    nc.vector.tensor_tensor(fR[:], fg[:, 0:FD_C],
                                            fg[:, FD_C:FDGC], OP.add)
                    hR = wk_pool.tile([P, FD_C], dt.bfloat16, tag="hR")
                    nc.vector.tensor_tensor(hR[:], hg[:, 0:FD_C],
                                            hg[:, FD_C:FDGC], OP.add)
                    h2R = wk_pool.tile([P, FD_C], dt.bfloat16, tag="h2R")
                    nc.vector.tensor_tensor(h2R[:], h2g[:, 0:FD_C],
                                            h2g[:, FD_C:FDGC], OP.add)

                    st = gi == 0
                    fin = gi == NGROUP_ - 1
                    for (acc, src) in ((p0, fR), (p1, hR), (p2, h2R)):
                        nc.tensor.matmul(acc[:, 0:512], ones[:],
                                         src[:, 0:512],
                                         start=st, stop=fin)
                        nc.tensor.matmul(acc[:, 512:FD_C], ones[:],
                                         src[:, 512:FD_C],
                                         start=st, stop=fin)
                    nc.tensor.matmul(pq[:], ones[:], qg[:],
                                     start=st, stop=fin)

            outt = cst_pool.tile([1, NPART], dt.float32, tag="out")
            nc.scalar.copy(outt[:, 0:FD_C], p0[:])
            nc.scalar.copy(outt[:, FD_C:2 * FD_C], p1[:])
            nc.scalar.copy(outt[:, 2 * FD_C:3 * FD_C], p2[:])
            nc.scalar.copy(outt[:, 3 * FD_C:NPART], pq[:])
            nc.sync.dma_start(po_d.ap(), outt[:])

    nc.compile()
    return nc


# ---------------------------------------------------------------------------
# Cached PJRT executor (same as v1)
# ---------------------------------------------------------------------------

_EXEC = None


def _get_executor():
    global _EXEC
    if _EXEC is not None:
        return _EXEC

    import jax
    import concourse.mybir as mybir
    from concourse import bass2jax
    from jax.sharding import Mesh, PartitionSpec
    from jax.experimental.shard_map import shard_map

    nc = build(1)
    bass2jax.install_neuronx_cc_hook()

    partition_name = (nc.partition_id_tensor.name
                      if nc.partition_id_tensor else None)
    in_names, out_names, out_avals = [], [], []
    for alloc in nc.m.functions[0].allocations:
        if not isinstance(alloc, mybir.MemoryLocationSet):
            continue
        name = alloc.memorylocations[0].name
        if alloc.kind == "ExternalInput":
            if name != partition_name:
                in_names.append(name)
        elif alloc.kind == "ExternalOutput":
            out_names.append(name)
            out_avals.append(jax.core.ShapedArray(
                tuple(alloc.tensor_shape), mybir.dt.np(alloc.dtype)))

    n_params = len(in_names)
    n_outs = len(out_avals)
    all_in_names = list(in_names) + list(out_names)
    if partition_name is not None:
        all_in_names.append(partition_name)

    def _body(*args):
        operands = list(args)
        if partition_name is not None:
            operands.append(bass2jax.partition_id_tensor())
        return tuple(bass2jax._bass_exec_p.bind(
            *operands,
            out_avals=tuple(out_avals),
            in_names=tuple(all_in_names),
            out_names=tuple(out_names),
            lowering_input_output_aliases=(),
            sim_require_finite=True,
            sim_require_nnan=True,
            nc=nc,
        ))

    devices = jax.devices()[:NCORES]
    mesh = Mesh(np.asarray(devices), ("core",))
    in_specs = (PartitionSpec("core"),) * (n_params + n_outs)
    out_specs = (PartitionSpec("core"),) * n_outs
    donate = tuple(range(n_params, n_params + n_outs))
    sharded = jax.jit(
        shard_map(_body, mesh=mesh, in_specs=in_specs, out_specs=out_specs,
                  check_rep=False),
        donate_argnums=donate, keep_unused=True)

    _EXEC = (sharded, in_names, out_names, out_avals)
    return _EXEC


def run_device_partials(output: np.ndarray, target: np.ndarray) -> np.ndarray:
    sharded, in_names, out_names, out_avals = _get_executor()
    feeds = {"output": np.ascontiguousarray(output, dtype=np.float32),
             "target": np.ascontiguousarray(target, dtype=np.float32)}
    ins = [feeds[n] for n in in_names]
    zeros = [np.zeros((NCORES * a.shape[0],) + a.shape[1:], a.dtype)
             for a in out_avals]
    outs = sharded(*ins, *zeros)
    idx = out_names.index("partials")
    return np.asarray(outs[idx]).reshape(NCORES, NPART)


def combine_partials(partials: np.ndarray,
                     binary_class_weights: np.ndarray) -> np.float32:
    p = partials.astype(np.float64).sum(axis=0)
    S0 = p[0:FD_C].reshape(T, 13).sum(axis=0)
    Sh = p[FD_C:2 * FD_C].reshape(T, 13).sum(axis=0)
    Sh2 = p[2 * FD_C:3 * FD_C].reshape(T, 13).sum(axis=0)
    Q = p[3 * FD_C:NPART].reshape(G_FIX * T, 3).sum(axis=0)
    w = np.asarray(binary_class_weights, dtype=np.float64)
    F1 = (Sh + Sh2) / 2.0
    F0 = S0 - Sh2
    focal = np.sum((1.0 - w) * F0 + w * F1)
    mse = Q / float(B)
    loss = 10.0 * mse[0] + mse[1] + mse[2] + focal
    return np.float32(loss)


def kernel(output: np.ndarray, target: np.ndarray,
           binary_class_weights: np.ndarray) -> np.ndarray:
    partials = run_device_partials(output, target)
    return np.asarray(combine_partials(partials, binary_class_weights))
